# revision 30
# baseline (speedup 1.0000x reference)
"""Averaged Hausdorff loss kernel for 8 Trainium2 NeuronCores.

Math: for row-sharded blocks, d2[i,j] = |s1_i|^2 + |s2_j|^2 - 2<s1_i, s2_j>
is computed as a single K=13 matmul with augmented operands.  Inputs are
split hi/lo into two fp16 parts (x = xh + xl exact to ~2^-22 rel), so the
matmul runs at full PE rate (1 cycle/row vs 4 for fp32) while keeping
~fp32 accuracy: the K rows pair up as
    xh*(-2yh) (3) + xh*(-2yl) (3) + xl*(-2yh) (3) + nh*1 + nl*1 + 1*n'h + 1*n'l
so each PSUM tile holds squared distances directly.  min_j sqrt(d2) =
sqrt(min_j d2), so only the [128,1] row-mins ever leave the device; the
sqrt + mean (or max) run on host in fp64.

Sharding: core r owns rows [r*1024, (r+1)*1024) of set1 (reduced against
all of set2) and the same rows of set2 (reduced against all of set1).
Both directions are free-dim row-min reductions; no collectives needed.
"""

import sys

sys.path.insert(0, "/opt/trn_rl_repo")

import numpy as np

N_CORES = 8
N = 8192          # set1 rows
M = 8192          # set2 rows
D = 3
ROWS_PER_CORE = N // N_CORES          # 1024
BLOCKS = ROWS_PER_CORE // 128         # 8 row-blocks of 128
CHUNK = 512                           # matmul free dim (one PSUM bank)
N_CHUNKS = M // CHUNK                 # 8 chunks per block row
K = 13                                # augmented contraction dim
FP32_MAX = 3.4e38

_compiled = None


def _enable_walrus_ldw_opt():
    """concourse pins --enable-ldw-opt=false; our inner loop issues 16
    matmuls per weight load, so redundant LDWEIGHTS cost ~34us of PE time.
    Rewrite the flag at the walrus_driver invocation boundary."""
    from concourse import bass_utils

    if getattr(bass_utils.run_command, "_ldw_patched", False):
        return
    orig = bass_utils.run_command

    def patched(argv, **kwargs):
        if argv and "walrus_driver" in str(argv[0]):
            argv = [
                "--enable-ldw-opt=true" if a == "--enable-ldw-opt=false" else a
                for a in argv
            ]
        return orig(argv, **kwargs)

    patched._ldw_patched = True
    bass_utils.run_command = patched


def _register_min2():
    """Register a custom fused DVE op (per-NEFF uop table):
    out[p,k] = min(in0[p,k], in1[p,k]); accum_out[p] = min(s0[p], min_k out[p,k]).
    One instruction consumes two fp32 streams (PSUM + SBUF ports) at
    2 elements/cycle — twice the throughput of stock tensor_reduce."""
    from concourse import dve_ops
    from concourse.dve_spec import Spec, Src0, Src1, minn, C0, AluOp, lower
    from concourse.dve_spec import _has_src1 as has_src1
    from concourse.dve_uop import DveOpSpec

    name = "MIN2_REDUCE_ANT"
    if name in dve_ops._SUB_OPCODE_FOR_NAME:
        return next(op for op in dve_ops.OPS if op.name == name)

    def _ref(in0, in1, c0, c1, c2):
        b = np.minimum(in0.astype(np.float32), in1.astype(np.float32))
        acc = np.minimum(
            np.asarray(c0, np.float32).reshape(-1, 1) if np.ndim(c0) else np.float32(c0),
            b.reshape(b.shape[0], -1).min(axis=-1, keepdims=True),
        )
        return b, acc

    spec = Spec(body=minn(Src0, Src1), accum=AluOp.MIN, accum_init=C0, reference=_ref)
    op = dve_ops.DveOp(name, spec, subdim=False, uops_sha={})
    dve_ops.OPS.append(op)
    dve_ops._SUB_OPCODE_FOR_NAME[name] = (
        dve_ops._CUSTOM_DVE_ROW_BASE + len(dve_ops.OPS) - 1
    )
    assert dve_ops._SUB_OPCODE_FOR_NAME[name] < 0x20
    dve_ops.CUSTOM_DVE_SPECS[name] = spec
    for ver in ("v3", "v4"):
        compiled = DveOpSpec(
            name=name,
            opcode=dve_ops.get_dve_sub_opcode(name),
            uops=lower(spec, ver=ver),
            rd1_en=has_src1(spec),
        )
        op.uops_sha[ver] = compiled.sha(ver)
    return op


def _build_program():
    import concourse.tile as tile
    from concourse.tile import add_dep_helper
    from concourse import bacc, mybir

    min2 = _register_min2()

    nc = bacc.Bacc("TRN2", target_bir_lowering=False, debug=False)
    f32 = mybir.dt.float32
    f16 = mybir.dt.float16

    lhs1_d = nc.dram_tensor("lhs1", [K, ROWS_PER_CORE], f16, kind="ExternalInput")
    rhs2_d = nc.dram_tensor("rhs2", [K, M], f16, kind="ExternalInput")
    lhs2_d = nc.dram_tensor("lhs2", [K, ROWS_PER_CORE], f16, kind="ExternalInput")
    rhs1_d = nc.dram_tensor("rhs1", [K, N], f16, kind="ExternalInput")
    out_d = nc.dram_tensor("out", [128, 2 * BLOCKS], f32, kind="ExternalOutput")

    with tile.TileContext(nc) as tc:
        with (
            tc.tile_pool(name="ops", bufs=1) as ops,
            tc.tile_pool(name="ps_keep", bufs=2, space="PSUM") as ps_keep,
            tc.tile_pool(name="ps_copy", bufs=2, space="PSUM") as ps_copy,
            tc.tile_pool(name="scopy", bufs=5) as scopy,
            tc.tile_pool(name="scratch", bufs=3) as scratch_pool,
            tc.tile_pool(name="small", bufs=1) as small,
        ):
            # Operand stacks replicated at base partitions 0 and 32 so
            # consecutive matmuls target different PE row-groups: LDWEIGHTS
            # for one row-group overlaps the matmul streaming in the other.
            # The first 2048 columns live in separate "early" tiles: tensor
            # dependencies are tile-granular, so this lets the first matmuls
            # start as soon as the small early DMAs land instead of waiting
            # for the full operand load.
            E = 4 * CHUNK
            lhs1 = ops.tile([32 + K, ROWS_PER_CORE], f16, tag="lhs1")
            lhs2 = ops.tile([32 + K, ROWS_PER_CORE], f16, tag="lhs2")
            rhs2e = ops.tile([32 + K, E], f16, tag="rhs2e")
            rhs2 = ops.tile([32 + K, M - E], f16, tag="rhs2")
            rhs1e = ops.tile([32 + K, E], f16, tag="rhs1e")
            rhs1 = ops.tile([32 + K, N - E], f16, tag="rhs1")
            for g in (0, 32):
                nc.sync.dma_start(lhs1[g : g + K, :], lhs1_d[:])
                nc.gpsimd.dma_start(lhs2[g : g + K, :], lhs2_d[:])
            for g in (0, 32):
                nc.sync.dma_start(rhs2e[g : g + K, :], rhs2_d[:, 0:E])
                nc.gpsimd.dma_start(rhs1e[g : g + K, :], rhs1_d[:, 0:E])
            for s in (slice(E, 4096), slice(4096, M)):
                d = slice(s.start - E, s.stop - E)
                for g in (0, 32):
                    nc.sync.dma_start(rhs2[g : g + K, d], rhs2_d[:, s])
                    nc.gpsimd.dma_start(rhs1[g : g + K, d], rhs1_d[:, s])

            rowmin = small.tile([128, 2 * BLOCKS], f32, tag="rowmin")
            prev_mm = None

            # One block in flight (PSUM double-buffers across quads), but
            # consecutive matmuls alternate between the two operand replicas
            # (PE row-groups 0 and 32), so pairs of matmuls stream
            # concurrently — 2 cols/cycle.  That matters because the HAM
            # clock never leaves 1.2 GHz in this environment.  Per quad: two
            # chunks land in a 2-bank PSUM tile the fused MIN2 op reads via
            # the PSUM port; the next two are copied to SBUF by ScalarE and
            # feed the SBUF port, so each MIN2 consumes 2 elements/cycle.
            # The block's running min chains through `cell`.
            for o, (lhs_s, rhs_e, rhs_r) in enumerate(
                ((lhs1, rhs2e, rhs2), (lhs2, rhs1e, rhs1))
            ):
                for b in range(BLOCKS):
                    cell = rowmin[:, o * BLOCKS + b : o * BLOCKS + b + 1]
                    bc = slice(b * 128, (b + 1) * 128)
                    for q in range(N_CHUNKS // 4):
                        rhs_s = rhs_e if q == 0 else rhs_r
                        j0 = 4 * q * CHUNK - (0 if q == 0 else E)
                        pk = ps_keep.tile([128, 2, CHUNK], f32, name="pk", tag="pk")
                        pc = ps_copy.tile([128, 2, CHUNK], f32, name="pc", tag="pc")
                        for t, dst in ((0, pk), (1, pk), (2, pc), (3, pc)):
                            g = 32 * (t % 2)
                            nc.tensor.matmul(
                                dst[:, t % 2, :],
                                lhs_s[g : g + K, bc],
                                rhs_s[g : g + K, j0 + t * CHUNK : j0 + (t + 1) * CHUNK],
                            )
                        sc = scopy.tile([128, 2, CHUNK], f32, name="sc", tag="sc")
                        nc.scalar.copy(sc[:], pc[:])
                        scr = scratch_pool.tile([128, 2, CHUNK], f32, name="scr", tag="scr")
                        nc.vector._custom_dve(
                            min2,
                            out=scr[:],
                            in0=pk[:],
                            in1=sc[:],
                            s0=FP32_MAX if q == 0 else cell,
                            accum_out=cell,
                        )

            nc.sync.dma_start(out_d[:], rowmin[:])

    nc.compile()
    return nc


def _get_program():
    global _compiled
    if _compiled is None:
        _compiled = _build_program()
    return _compiled


def _split16(v):
    """fp64 vector -> (hi, lo) fp16 with v ~= hi + lo to ~2^-22 rel."""
    hi = v.astype(np.float16)
    lo = (v - hi.astype(np.float64)).astype(np.float16)
    return hi.astype(np.float64), lo.astype(np.float64)


def _aug_operands(s):
    """Build [13, n] lhsT and rhs operand stacks in fp16 (hi/lo split)."""
    s64 = s.astype(np.float64)
    n = (s64 * s64).sum(axis=1)
    ones = np.ones(s.shape[0], dtype=np.float64)
    xh = [None] * D
    xl = [None] * D
    for d in range(D):
        xh[d], xl[d] = _split16(s64[:, d])
    nh, nl = _split16(n)
    lhs = np.stack(
        [xh[0], xh[1], xh[2], xh[0], xh[1], xh[2], xl[0], xl[1], xl[2],
         nh, nl, ones, ones]
    ).astype(np.float16)
    rhs = np.stack(
        [-2 * xh[0], -2 * xh[1], -2 * xh[2], -2 * xl[0], -2 * xl[1], -2 * xl[2],
         -2 * xh[0], -2 * xh[1], -2 * xh[2], ones, ones, nh, nl]
    ).astype(np.float16)
    return np.ascontiguousarray(lhs), np.ascontiguousarray(rhs)


def _run_device(s1, s2, trace=False):
    from concourse.bass_utils import run_bass_kernel_spmd

    nc = _get_program()
    lhs1_full, rhs1_full = _aug_operands(s1)
    lhs2_full, rhs2_full = _aug_operands(s2)

    in_maps = []
    for r in range(N_CORES):
        sl = slice(r * ROWS_PER_CORE, (r + 1) * ROWS_PER_CORE)
        in_maps.append(
            {
                "lhs1": np.ascontiguousarray(lhs1_full[:, sl]),
                "rhs2": rhs2_full,
                "lhs2": np.ascontiguousarray(lhs2_full[:, sl]),
                "rhs1": rhs1_full,
            }
        )

    res = run_bass_kernel_spmd(nc, in_maps, list(range(N_CORES)), trace=trace)

    d1min = np.concatenate(
        [res.results[r]["out"][:, 0:BLOCKS].T.reshape(-1) for r in range(N_CORES)]
    )
    d2min = np.concatenate(
        [res.results[r]["out"][:, BLOCKS : 2 * BLOCKS].T.reshape(-1) for r in range(N_CORES)]
    )
    return d1min, d2min, res


def kernel(set1, set2, hausdorff=0, w_set1_set2=1, w_set2_set1=1, n_outputs=1):
    s1 = np.ascontiguousarray(np.asarray(set1, dtype=np.float32))
    s2 = np.ascontiguousarray(np.asarray(set2, dtype=np.float32))
    assert s1.shape == (N, D) and s2.shape == (M, D), (s1.shape, s2.shape)
    hausdorff = int(np.asarray(hausdorff))
    w12 = int(np.asarray(w_set1_set2))
    w21 = int(np.asarray(w_set2_set1))
    n_outputs = int(np.asarray(n_outputs))

    d1min, d2min, _ = _run_device(s1, s2)

    d1 = np.sqrt(np.maximum(d1min, 0.0).astype(np.float64))
    d2 = np.sqrt(np.maximum(d2min, 0.0).astype(np.float64))
    reduce = np.mean if hausdorff == 0 else np.max
    t12 = np.float32(reduce(d1)) if w12 != 0 else np.float32(0.0)
    t21 = np.float32(reduce(d2)) if w21 != 0 else np.float32(0.0)

    if n_outputs == 1:
        return np.float32(t12 + t21)
    return (t12, t21)


# revision 33
# speedup vs baseline: 1.0053x; 1.0053x over previous
"""Averaged Hausdorff loss kernel for 8 Trainium2 NeuronCores.

Math: for row-sharded blocks, d2[i,j] = |s1_i|^2 + |s2_j|^2 - 2<s1_i, s2_j>
is computed as a single K=13 matmul with augmented operands.  Inputs are
split hi/lo into two fp16 parts (x = xh + xl exact to ~2^-22 rel), so the
matmul runs at full PE rate (1 cycle/row vs 4 for fp32) while keeping
~fp32 accuracy: the K rows pair up as
    xh*(-2yh) (3) + xh*(-2yl) (3) + xl*(-2yh) (3) + nh*1 + nl*1 + 1*n'h + 1*n'l
so each PSUM tile holds squared distances directly.  min_j sqrt(d2) =
sqrt(min_j d2), so only the [128,1] row-mins ever leave the device; the
sqrt + mean (or max) run on host in fp64.

Sharding: core r owns rows [r*1024, (r+1)*1024) of set1 (reduced against
all of set2) and the same rows of set2 (reduced against all of set1).
Both directions are free-dim row-min reductions; no collectives needed.
"""

import sys

sys.path.insert(0, "/opt/trn_rl_repo")

import numpy as np

N_CORES = 8
N = 8192          # set1 rows
M = 8192          # set2 rows
D = 3
ROWS_PER_CORE = N // N_CORES          # 1024
BLOCKS = ROWS_PER_CORE // 128         # 8 row-blocks of 128
CHUNK = 512                           # matmul free dim (one PSUM bank)
N_CHUNKS = M // CHUNK                 # 8 chunks per block row
K = 13                                # augmented contraction dim
FP32_MAX = 3.4e38

_compiled = None


def _enable_walrus_ldw_opt():
    """concourse pins --enable-ldw-opt=false; our inner loop issues 16
    matmuls per weight load, so redundant LDWEIGHTS cost ~34us of PE time.
    Rewrite the flag at the walrus_driver invocation boundary."""
    from concourse import bass_utils

    if getattr(bass_utils.run_command, "_ldw_patched", False):
        return
    orig = bass_utils.run_command

    def patched(argv, **kwargs):
        if argv and "walrus_driver" in str(argv[0]):
            argv = [
                "--enable-ldw-opt=true" if a == "--enable-ldw-opt=false" else a
                for a in argv
            ]
        return orig(argv, **kwargs)

    patched._ldw_patched = True
    bass_utils.run_command = patched


def _register_min2():
    """Register a custom fused DVE op (per-NEFF uop table):
    out[p,k] = min(in0[p,k], in1[p,k]); accum_out[p] = min(s0[p], min_k out[p,k]).
    One instruction consumes two fp32 streams (PSUM + SBUF ports) at
    2 elements/cycle — twice the throughput of stock tensor_reduce."""
    from concourse import dve_ops
    from concourse.dve_spec import Spec, Src0, Src1, minn, C0, AluOp, lower
    from concourse.dve_spec import _has_src1 as has_src1
    from concourse.dve_uop import DveOpSpec

    name = "MIN2_REDUCE_ANT"
    if name in dve_ops._SUB_OPCODE_FOR_NAME:
        return next(op for op in dve_ops.OPS if op.name == name)

    def _ref(in0, in1, c0, c1, c2):
        b = np.minimum(in0.astype(np.float32), in1.astype(np.float32))
        acc = np.minimum(
            np.asarray(c0, np.float32).reshape(-1, 1) if np.ndim(c0) else np.float32(c0),
            b.reshape(b.shape[0], -1).min(axis=-1, keepdims=True),
        )
        return b, acc

    spec = Spec(body=minn(Src0, Src1), accum=AluOp.MIN, accum_init=C0, reference=_ref)
    op = dve_ops.DveOp(name, spec, subdim=False, uops_sha={})
    dve_ops.OPS.append(op)
    dve_ops._SUB_OPCODE_FOR_NAME[name] = (
        dve_ops._CUSTOM_DVE_ROW_BASE + len(dve_ops.OPS) - 1
    )
    assert dve_ops._SUB_OPCODE_FOR_NAME[name] < 0x20
    dve_ops.CUSTOM_DVE_SPECS[name] = spec
    for ver in ("v3", "v4"):
        compiled = DveOpSpec(
            name=name,
            opcode=dve_ops.get_dve_sub_opcode(name),
            uops=lower(spec, ver=ver),
            rd1_en=has_src1(spec),
        )
        op.uops_sha[ver] = compiled.sha(ver)
    return op


def _build_program():
    import concourse.tile as tile
    from concourse.tile import add_dep_helper
    from concourse import bacc, mybir

    min2 = _register_min2()

    nc = bacc.Bacc("TRN2", target_bir_lowering=False, debug=False)
    f32 = mybir.dt.float32
    f16 = mybir.dt.float16

    lhs1_d = nc.dram_tensor("lhs1", [K, ROWS_PER_CORE], f16, kind="ExternalInput")
    rhs2_d = nc.dram_tensor("rhs2", [K, M], f16, kind="ExternalInput")
    lhs2_d = nc.dram_tensor("lhs2", [K, ROWS_PER_CORE], f16, kind="ExternalInput")
    rhs1_d = nc.dram_tensor("rhs1", [K, N], f16, kind="ExternalInput")
    out_d = nc.dram_tensor("out", [128, 2 * BLOCKS], f32, kind="ExternalOutput")

    with tile.TileContext(nc) as tc:
        with (
            tc.tile_pool(name="ops", bufs=1) as ops,
            tc.tile_pool(name="ps_keep", bufs=2, space="PSUM") as ps_keep,
            tc.tile_pool(name="ps_copy", bufs=2, space="PSUM") as ps_copy,
            tc.tile_pool(name="scopy", bufs=5) as scopy,
            tc.tile_pool(name="scratch", bufs=3) as scratch_pool,
            tc.tile_pool(name="small", bufs=1) as small,
        ):
            # Operand stacks replicated at base partitions 0 and 32 so
            # consecutive matmuls target different PE row-groups: LDWEIGHTS
            # for one row-group overlaps the matmul streaming in the other.
            # The first 2048 columns live in separate "early" tiles: tensor
            # dependencies are tile-granular, so this lets the first matmuls
            # start as soon as the small early DMAs land instead of waiting
            # for the full operand load.
            E = 4 * CHUNK
            lhs1 = ops.tile([32 + K, ROWS_PER_CORE], f16, tag="lhs1")
            lhs2 = ops.tile([32 + K, ROWS_PER_CORE], f16, tag="lhs2")
            rhs2e = ops.tile([32 + K, E], f16, tag="rhs2e")
            rhs2 = ops.tile([32 + K, M - E], f16, tag="rhs2")
            rhs1e = ops.tile([32 + K, E], f16, tag="rhs1e")
            rhs1 = ops.tile([32 + K, N - E], f16, tag="rhs1")
            # Early-critical pieces go on the otherwise-idle vector/scalar
            # DMA queues: consumers wait on per-queue completion counts, so
            # keeping these queues short lets the first matmuls start as
            # soon as their own data lands.
            for g in (0, 32):
                nc.scalar.dma_start(lhs1[g : g + K, :], lhs1_d[:])
                nc.scalar.dma_start(rhs2e[g : g + K, :], rhs2_d[:, 0:E])
                nc.gpsimd.dma_start(lhs2[g : g + K, :], lhs2_d[:])
                nc.gpsimd.dma_start(rhs1e[g : g + K, :], rhs1_d[:, 0:E])
            for s in (slice(E, 4096), slice(4096, M)):
                d = slice(s.start - E, s.stop - E)
                for g in (0, 32):
                    nc.sync.dma_start(rhs2[g : g + K, d], rhs2_d[:, s])
                    nc.sync.dma_start(rhs1[g : g + K, d], rhs1_d[:, s])

            rowmin = small.tile([128, 2 * BLOCKS], f32, tag="rowmin")
            prev_mm = None

            # One block in flight (PSUM double-buffers across quads), but
            # consecutive matmuls alternate between the two operand replicas
            # (PE row-groups 0 and 32), so pairs of matmuls stream
            # concurrently — 2 cols/cycle.  That matters because the HAM
            # clock never leaves 1.2 GHz in this environment.  Per quad: two
            # chunks land in a 2-bank PSUM tile the fused MIN2 op reads via
            # the PSUM port; the next two are copied to SBUF by ScalarE and
            # feed the SBUF port, so each MIN2 consumes 2 elements/cycle.
            # The block's running min chains through `cell`.
            for o, (lhs_s, rhs_e, rhs_r) in enumerate(
                ((lhs1, rhs2e, rhs2), (lhs2, rhs1e, rhs1))
            ):
                for b in range(BLOCKS):
                    cell = rowmin[:, o * BLOCKS + b : o * BLOCKS + b + 1]
                    bc = slice(b * 128, (b + 1) * 128)
                    for q in range(N_CHUNKS // 4):
                        rhs_s = rhs_e if q == 0 else rhs_r
                        j0 = 4 * q * CHUNK - (0 if q == 0 else E)
                        pk = ps_keep.tile([128, 2, CHUNK], f32, name="pk", tag="pk")
                        pc = ps_copy.tile([128, 2, CHUNK], f32, name="pc", tag="pc")
                        for t, dst in ((0, pk), (1, pk), (2, pc), (3, pc)):
                            g = 32 * (t % 2)
                            nc.tensor.matmul(
                                dst[:, t % 2, :],
                                lhs_s[g : g + K, bc],
                                rhs_s[g : g + K, j0 + t * CHUNK : j0 + (t + 1) * CHUNK],
                            )
                        sc = scopy.tile([128, 2, CHUNK], f32, name="sc", tag="sc")
                        nc.scalar.copy(sc[:], pc[:])
                        scr = scratch_pool.tile([128, 2, CHUNK], f32, name="scr", tag="scr")
                        nc.vector._custom_dve(
                            min2,
                            out=scr[:],
                            in0=pk[:],
                            in1=sc[:],
                            s0=FP32_MAX if q == 0 else cell,
                            accum_out=cell,
                        )

            nc.sync.dma_start(out_d[:], rowmin[:])

    nc.compile()
    return nc


def _get_program():
    global _compiled
    if _compiled is None:
        _compiled = _build_program()
    return _compiled


def _split16(v):
    """fp64 vector -> (hi, lo) fp16 with v ~= hi + lo to ~2^-22 rel."""
    hi = v.astype(np.float16)
    lo = (v - hi.astype(np.float64)).astype(np.float16)
    return hi.astype(np.float64), lo.astype(np.float64)


def _aug_operands(s):
    """Build [13, n] lhsT and rhs operand stacks in fp16 (hi/lo split)."""
    s64 = s.astype(np.float64)
    n = (s64 * s64).sum(axis=1)
    ones = np.ones(s.shape[0], dtype=np.float64)
    xh = [None] * D
    xl = [None] * D
    for d in range(D):
        xh[d], xl[d] = _split16(s64[:, d])
    nh, nl = _split16(n)
    lhs = np.stack(
        [xh[0], xh[1], xh[2], xh[0], xh[1], xh[2], xl[0], xl[1], xl[2],
         nh, nl, ones, ones]
    ).astype(np.float16)
    rhs = np.stack(
        [-2 * xh[0], -2 * xh[1], -2 * xh[2], -2 * xl[0], -2 * xl[1], -2 * xl[2],
         -2 * xh[0], -2 * xh[1], -2 * xh[2], ones, ones, nh, nl]
    ).astype(np.float16)
    return np.ascontiguousarray(lhs), np.ascontiguousarray(rhs)


def _run_device(s1, s2, trace=False):
    from concourse.bass_utils import run_bass_kernel_spmd

    nc = _get_program()
    lhs1_full, rhs1_full = _aug_operands(s1)
    lhs2_full, rhs2_full = _aug_operands(s2)

    in_maps = []
    for r in range(N_CORES):
        sl = slice(r * ROWS_PER_CORE, (r + 1) * ROWS_PER_CORE)
        in_maps.append(
            {
                "lhs1": np.ascontiguousarray(lhs1_full[:, sl]),
                "rhs2": rhs2_full,
                "lhs2": np.ascontiguousarray(lhs2_full[:, sl]),
                "rhs1": rhs1_full,
            }
        )

    res = run_bass_kernel_spmd(nc, in_maps, list(range(N_CORES)), trace=trace)

    d1min = np.concatenate(
        [res.results[r]["out"][:, 0:BLOCKS].T.reshape(-1) for r in range(N_CORES)]
    )
    d2min = np.concatenate(
        [res.results[r]["out"][:, BLOCKS : 2 * BLOCKS].T.reshape(-1) for r in range(N_CORES)]
    )
    return d1min, d2min, res


def kernel(set1, set2, hausdorff=0, w_set1_set2=1, w_set2_set1=1, n_outputs=1):
    s1 = np.ascontiguousarray(np.asarray(set1, dtype=np.float32))
    s2 = np.ascontiguousarray(np.asarray(set2, dtype=np.float32))
    assert s1.shape == (N, D) and s2.shape == (M, D), (s1.shape, s2.shape)
    hausdorff = int(np.asarray(hausdorff))
    w12 = int(np.asarray(w_set1_set2))
    w21 = int(np.asarray(w_set2_set1))
    n_outputs = int(np.asarray(n_outputs))

    d1min, d2min, _ = _run_device(s1, s2)

    d1 = np.sqrt(np.maximum(d1min, 0.0).astype(np.float64))
    d2 = np.sqrt(np.maximum(d2min, 0.0).astype(np.float64))
    reduce = np.mean if hausdorff == 0 else np.max
    t12 = np.float32(reduce(d1)) if w12 != 0 else np.float32(0.0)
    t21 = np.float32(reduce(d2)) if w21 != 0 else np.float32(0.0)

    if n_outputs == 1:
        return np.float32(t12 + t21)
    return (t12, t21)


# revision 39
# speedup vs baseline: 1.1191x; 1.1132x over previous
"""Averaged Hausdorff loss kernel for 8 Trainium2 NeuronCores.

Math: for row-sharded blocks, d2[i,j] = |s1_i|^2 + |s2_j|^2 - 2<s1_i, s2_j>
is computed as a single K=13 matmul with augmented operands.  Inputs are
split hi/lo into two fp16 parts (x = xh + xl exact to ~2^-22 rel), so the
matmul runs at full PE rate (1 cycle/row vs 4 for fp32) while keeping
~fp32 accuracy: the K rows pair up as
    xh*(-2yh) (3) + xh*(-2yl) (3) + xl*(-2yh) (3) + nh*1 + nl*1 + 1*n'h + 1*n'l
so each PSUM tile holds squared distances directly.  min_j sqrt(d2) =
sqrt(min_j d2), so only the [128,1] row-mins ever leave the device; the
sqrt + mean (or max) run on host in fp64.

Sharding: core r owns rows [r*1024, (r+1)*1024) of set1 (reduced against
all of set2) and the same rows of set2 (reduced against all of set1).
Both directions are free-dim row-min reductions; no collectives needed.
"""

import sys

sys.path.insert(0, "/opt/trn_rl_repo")

import numpy as np

N_CORES = 8
N = 8192          # set1 rows
M = 8192          # set2 rows
D = 3
ROWS_PER_CORE = N // N_CORES          # 1024
BLOCKS = ROWS_PER_CORE // 128         # 8 row-blocks of 128
CHUNK = 512                           # matmul free dim (one PSUM bank)
N_CHUNKS = M // CHUNK                 # 8 chunks per block row
K = 13                                # augmented contraction dim
FP32_MAX = 3.4e38

_compiled = None


def _enable_walrus_ldw_opt():
    """concourse pins --enable-ldw-opt=false; our inner loop issues 16
    matmuls per weight load, so redundant LDWEIGHTS cost ~34us of PE time.
    Rewrite the flag at the walrus_driver invocation boundary."""
    from concourse import bass_utils

    if getattr(bass_utils.run_command, "_ldw_patched", False):
        return
    orig = bass_utils.run_command

    def patched(argv, **kwargs):
        if argv and "walrus_driver" in str(argv[0]):
            argv = [
                "--enable-ldw-opt=true" if a == "--enable-ldw-opt=false" else a
                for a in argv
            ]
        return orig(argv, **kwargs)

    patched._ldw_patched = True
    bass_utils.run_command = patched


def _register_min2():
    """Register a custom fused DVE op (per-NEFF uop table):
    out[p,k] = min(in0[p,k], in1[p,k]); accum_out[p] = min(s0[p], min_k out[p,k]).
    One instruction consumes two fp32 streams (PSUM + SBUF ports) at
    2 elements/cycle — twice the throughput of stock tensor_reduce."""
    from concourse import dve_ops
    from concourse.dve_spec import Spec, Src0, Src1, minn, C0, AluOp, lower
    from concourse.dve_spec import _has_src1 as has_src1
    from concourse.dve_uop import DveOpSpec

    def _ref(in0, in1, c0, c1, c2):
        b = np.minimum(in0.astype(np.float32), in1.astype(np.float32))
        acc = np.minimum(
            np.asarray(c0, np.float32).reshape(-1, 1) if np.ndim(c0) else np.float32(c0),
            b.reshape(b.shape[0], -1).min(axis=-1, keepdims=True),
        )
        return b, acc

    ops_out = []
    for name, spec in (
        (
            "MIN2_REDUCE_ANT",
            Spec(body=minn(Src0, Src1), accum=AluOp.MIN, accum_init=C0,
                 reference=_ref),
        ),
    ):
        if name in dve_ops._SUB_OPCODE_FOR_NAME:
            ops_out.append(next(op for op in dve_ops.OPS if op.name == name))
            continue
        op = dve_ops.DveOp(name, spec, subdim=False, uops_sha={})
        dve_ops.OPS.append(op)
        dve_ops._SUB_OPCODE_FOR_NAME[name] = (
            dve_ops._CUSTOM_DVE_ROW_BASE + len(dve_ops.OPS) - 1
        )
        assert dve_ops._SUB_OPCODE_FOR_NAME[name] < 0x20
        dve_ops.CUSTOM_DVE_SPECS[name] = spec
        for ver in ("v3", "v4"):
            compiled = DveOpSpec(
                name=name,
                opcode=dve_ops.get_dve_sub_opcode(name),
                uops=lower(spec, ver=ver),
                rd1_en=has_src1(spec),
            )
            op.uops_sha[ver] = compiled.sha(ver)
        ops_out.append(op)
    return ops_out[0]


def _build_program():
    import concourse.tile as tile
    from concourse.tile import add_dep_helper
    from concourse import bacc, mybir

    min2 = _register_min2()

    nc = bacc.Bacc("TRN2", target_bir_lowering=False, debug=False)
    f32 = mybir.dt.float32
    f16 = mybir.dt.float16

    lhs1_d = nc.dram_tensor("lhs1", [K, ROWS_PER_CORE], f16, kind="ExternalInput")
    rhs2_d = nc.dram_tensor("rhs2", [K, M], f16, kind="ExternalInput")
    lhs2_d = nc.dram_tensor("lhs2", [K, ROWS_PER_CORE], f16, kind="ExternalInput")
    rhs1_d = nc.dram_tensor("rhs1", [K, N], f16, kind="ExternalInput")
    out_d = nc.dram_tensor("out", [128, 2 * BLOCKS], f32, kind="ExternalOutput")

    with tile.TileContext(nc) as tc:
        with (
            tc.tile_pool(name="ops", bufs=1) as ops,
            tc.tile_pool(name="ps_keep", bufs=2, space="PSUM") as ps_keep,
            tc.tile_pool(name="ps_copy", bufs=2, space="PSUM") as ps_copy,
            tc.tile_pool(name="scopy", bufs=5) as scopy,
            tc.tile_pool(name="scratch", bufs=3) as scratch_pool,
            tc.tile_pool(name="small", bufs=1) as small,
        ):
            # Operand stacks replicated at base partitions 0 and 32 so
            # consecutive matmuls target different PE row-groups: LDWEIGHTS
            # for one row-group overlaps the matmul streaming in the other.
            # The first 2048 columns live in separate "early" tiles: tensor
            # dependencies are tile-granular, so this lets the first matmuls
            # start as soon as the small early DMAs land instead of waiting
            # for the full operand load.
            E = 4 * CHUNK
            lhs1 = ops.tile([32 + K, ROWS_PER_CORE], f16, tag="lhs1")
            lhs2 = ops.tile([32 + K, ROWS_PER_CORE], f16, tag="lhs2")
            rhs2e = ops.tile([32 + K, E], f16, tag="rhs2e")
            rhs2 = ops.tile([32 + K, M - E], f16, tag="rhs2")
            rhs1e = ops.tile([32 + K, E], f16, tag="rhs1e")
            rhs1 = ops.tile([32 + K, N - E], f16, tag="rhs1")
            # Early-critical pieces go on the otherwise-idle vector/scalar
            # DMA queues: consumers wait on per-queue completion counts, so
            # keeping these queues short lets the first matmuls start as
            # soon as their own data lands.
            for g in (0, 32):
                nc.scalar.dma_start(lhs1[g : g + K, :], lhs1_d[:])
                nc.scalar.dma_start(rhs2e[g : g + K, :], rhs2_d[:, 0:E])
                nc.gpsimd.dma_start(lhs2[g : g + K, :], lhs2_d[:])
                nc.gpsimd.dma_start(rhs1e[g : g + K, :], rhs1_d[:, 0:E])
            for s in (slice(E, 4096), slice(4096, M)):
                d = slice(s.start - E, s.stop - E)
                for g in (0, 32):
                    nc.sync.dma_start(rhs2[g : g + K, d], rhs2_d[:, s])
                    nc.sync.dma_start(rhs1[g : g + K, d], rhs1_d[:, s])

            rowmin = small.tile([128, 2 * BLOCKS], f32, tag="rowmin")
            rowpart = small.tile([128, 2 * BLOCKS, N_CHUNKS // 4], f32, tag="rowpart")

            # One block in flight (PSUM double-buffers across quads), but
            # consecutive matmuls alternate between the two operand replicas
            # (PE row-groups 0 and 32), so pairs of matmuls stream
            # concurrently — 2 cols/cycle.  That matters because the HAM
            # clock never leaves 1.2 GHz in this environment.  Per quad: two
            # chunks land in a 2-bank PSUM tile the fused MIN2 op reads via
            # the PSUM port; the next two are copied to SBUF by ScalarE and
            # feed the SBUF port, so each MIN2 consumes 2 elements/cycle.
            # The block's running min chains through `cell`.
            for o, (lhs_s, rhs_e, rhs_r) in enumerate(
                ((lhs1, rhs2e, rhs2), (lhs2, rhs1e, rhs1))
            ):
                for b in range(BLOCKS):
                    ob = o * BLOCKS + b
                    bc = slice(b * 128, (b + 1) * 128)
                    for q in range(N_CHUNKS // 4):
                        rhs_s = rhs_e if q == 0 else rhs_r
                        j0 = 4 * q * CHUNK - (0 if q == 0 else E)
                        pk = ps_keep.tile([128, 2, CHUNK], f32, name="pk", tag="pk")
                        pc = ps_copy.tile([128, 2, CHUNK], f32, name="pc", tag="pc")
                        for t, dst in ((0, pk), (1, pk), (2, pc), (3, pc)):
                            g = 32 * (t % 2)
                            nc.tensor.matmul(
                                dst[:, t % 2, :],
                                lhs_s[g : g + K, bc],
                                rhs_s[g : g + K, j0 + t * CHUNK : j0 + (t + 1) * CHUNK],
                            )
                        sc = scopy.tile([128, 2, CHUNK], f32, name="sc", tag="sc")
                        nc.scalar.copy(sc[:], pc[:])
                        scr = scratch_pool.tile([128, 2, CHUNK], f32, name="scr", tag="scr")
                        nc.vector._custom_dve(
                            min2,
                            out=scr[:],
                            in0=pk[:],
                            in1=sc[:],
                            s0=FP32_MAX,
                            accum_out=rowpart[:, ob, q : q + 1],
                        )

            nc.vector.tensor_reduce(
                rowmin[:],
                rowpart[:],
                axis=mybir.AxisListType.X,
                op=mybir.AluOpType.min,
            )
            nc.sync.dma_start(out_d[:], rowmin[:])

    nc.compile()
    return nc


def _get_program():
    global _compiled
    if _compiled is None:
        _compiled = _build_program()
    return _compiled


def _split16(v):
    """fp64 vector -> (hi, lo) fp16 with v ~= hi + lo to ~2^-22 rel."""
    hi = v.astype(np.float16)
    lo = (v - hi.astype(np.float64)).astype(np.float16)
    return hi.astype(np.float64), lo.astype(np.float64)


def _aug_operands(s):
    """Build [13, n] lhsT and rhs operand stacks in fp16 (hi/lo split)."""
    s64 = s.astype(np.float64)
    n = (s64 * s64).sum(axis=1)
    ones = np.ones(s.shape[0], dtype=np.float64)
    xh = [None] * D
    xl = [None] * D
    for d in range(D):
        xh[d], xl[d] = _split16(s64[:, d])
    nh, nl = _split16(n)
    lhs = np.stack(
        [xh[0], xh[1], xh[2], xh[0], xh[1], xh[2], xl[0], xl[1], xl[2],
         nh, nl, ones, ones]
    ).astype(np.float16)
    rhs = np.stack(
        [-2 * xh[0], -2 * xh[1], -2 * xh[2], -2 * xl[0], -2 * xl[1], -2 * xl[2],
         -2 * xh[0], -2 * xh[1], -2 * xh[2], ones, ones, nh, nl]
    ).astype(np.float16)
    return np.ascontiguousarray(lhs), np.ascontiguousarray(rhs)


def _run_device(s1, s2, trace=False):
    from concourse.bass_utils import run_bass_kernel_spmd

    nc = _get_program()
    lhs1_full, rhs1_full = _aug_operands(s1)
    lhs2_full, rhs2_full = _aug_operands(s2)

    in_maps = []
    for r in range(N_CORES):
        sl = slice(r * ROWS_PER_CORE, (r + 1) * ROWS_PER_CORE)
        in_maps.append(
            {
                "lhs1": np.ascontiguousarray(lhs1_full[:, sl]),
                "rhs2": rhs2_full,
                "lhs2": np.ascontiguousarray(lhs2_full[:, sl]),
                "rhs1": rhs1_full,
            }
        )

    res = run_bass_kernel_spmd(nc, in_maps, list(range(N_CORES)), trace=trace)

    d1min = np.concatenate(
        [res.results[r]["out"][:, 0:BLOCKS].T.reshape(-1) for r in range(N_CORES)]
    )
    d2min = np.concatenate(
        [res.results[r]["out"][:, BLOCKS : 2 * BLOCKS].T.reshape(-1) for r in range(N_CORES)]
    )
    return d1min, d2min, res


def kernel(set1, set2, hausdorff=0, w_set1_set2=1, w_set2_set1=1, n_outputs=1):
    s1 = np.ascontiguousarray(np.asarray(set1, dtype=np.float32))
    s2 = np.ascontiguousarray(np.asarray(set2, dtype=np.float32))
    assert s1.shape == (N, D) and s2.shape == (M, D), (s1.shape, s2.shape)
    hausdorff = int(np.asarray(hausdorff))
    w12 = int(np.asarray(w_set1_set2))
    w21 = int(np.asarray(w_set2_set1))
    n_outputs = int(np.asarray(n_outputs))

    d1min, d2min, _ = _run_device(s1, s2)

    d1 = np.sqrt(np.maximum(d1min, 0.0).astype(np.float64))
    d2 = np.sqrt(np.maximum(d2min, 0.0).astype(np.float64))
    reduce = np.mean if hausdorff == 0 else np.max
    t12 = np.float32(reduce(d1)) if w12 != 0 else np.float32(0.0)
    t21 = np.float32(reduce(d2)) if w21 != 0 else np.float32(0.0)

    if n_outputs == 1:
        return np.float32(t12 + t21)
    return (t12, t21)


# revision 42
# speedup vs baseline: 1.1219x; 1.0025x over previous
"""Averaged Hausdorff loss kernel for 8 Trainium2 NeuronCores.

Math: for row-sharded blocks, d2[i,j] = |s1_i|^2 + |s2_j|^2 - 2<s1_i, s2_j>
is computed as a single K=13 matmul with augmented operands.  Inputs are
split hi/lo into two fp16 parts (x = xh + xl exact to ~2^-22 rel), so the
matmul runs at full PE rate (1 cycle/row vs 4 for fp32) while keeping
~fp32 accuracy: the K rows pair up as
    xh*(-2yh) (3) + xh*(-2yl) (3) + xl*(-2yh) (3) + nh*1 + nl*1 + 1*n'h + 1*n'l
so each PSUM tile holds squared distances directly.  min_j sqrt(d2) =
sqrt(min_j d2), so only the [128,1] row-mins ever leave the device; the
sqrt + mean (or max) run on host in fp64.

Sharding: core r owns rows [r*1024, (r+1)*1024) of set1 (reduced against
all of set2) and the same rows of set2 (reduced against all of set1).
Both directions are free-dim row-min reductions; no collectives needed.
"""

import sys

sys.path.insert(0, "/opt/trn_rl_repo")

import numpy as np

N_CORES = 8
N = 8192          # set1 rows
M = 8192          # set2 rows
D = 3
ROWS_PER_CORE = N // N_CORES          # 1024
BLOCKS = ROWS_PER_CORE // 128         # 8 row-blocks of 128
CHUNK = 512                           # matmul free dim (one PSUM bank)
N_CHUNKS = M // CHUNK                 # 8 chunks per block row
K = 13                                # augmented contraction dim
FP32_MAX = 3.4e38

_compiled = None


def _enable_walrus_ldw_opt():
    """concourse pins --enable-ldw-opt=false; our inner loop issues 16
    matmuls per weight load, so redundant LDWEIGHTS cost ~34us of PE time.
    Rewrite the flag at the walrus_driver invocation boundary."""
    from concourse import bass_utils

    if getattr(bass_utils.run_command, "_ldw_patched", False):
        return
    orig = bass_utils.run_command

    def patched(argv, **kwargs):
        if argv and "walrus_driver" in str(argv[0]):
            argv = [
                "--enable-ldw-opt=true" if a == "--enable-ldw-opt=false" else a
                for a in argv
            ]
        return orig(argv, **kwargs)

    patched._ldw_patched = True
    bass_utils.run_command = patched


def _register_min2():
    """Register a custom fused DVE op (per-NEFF uop table):
    out[p,k] = min(in0[p,k], in1[p,k]); accum_out[p] = min(s0[p], min_k out[p,k]).
    One instruction consumes two fp32 streams (PSUM + SBUF ports) at
    2 elements/cycle — twice the throughput of stock tensor_reduce."""
    from concourse import dve_ops
    from concourse.dve_spec import Spec, Src0, Src1, minn, C0, AluOp, lower
    from concourse.dve_spec import _has_src1 as has_src1
    from concourse.dve_uop import DveOpSpec

    def _ref(in0, in1, c0, c1, c2):
        b = np.minimum(in0.astype(np.float32), in1.astype(np.float32))
        acc = np.minimum(
            np.asarray(c0, np.float32).reshape(-1, 1) if np.ndim(c0) else np.float32(c0),
            b.reshape(b.shape[0], -1).min(axis=-1, keepdims=True),
        )
        return b, acc

    ops_out = []
    for name, spec in (
        (
            "MIN2_REDUCE_ANT",
            Spec(body=minn(Src0, Src1), accum=AluOp.MIN, accum_init=C0,
                 reference=_ref),
        ),
    ):
        if name in dve_ops._SUB_OPCODE_FOR_NAME:
            ops_out.append(next(op for op in dve_ops.OPS if op.name == name))
            continue
        op = dve_ops.DveOp(name, spec, subdim=False, uops_sha={})
        dve_ops.OPS.append(op)
        dve_ops._SUB_OPCODE_FOR_NAME[name] = (
            dve_ops._CUSTOM_DVE_ROW_BASE + len(dve_ops.OPS) - 1
        )
        assert dve_ops._SUB_OPCODE_FOR_NAME[name] < 0x20
        dve_ops.CUSTOM_DVE_SPECS[name] = spec
        for ver in ("v3", "v4"):
            compiled = DveOpSpec(
                name=name,
                opcode=dve_ops.get_dve_sub_opcode(name),
                uops=lower(spec, ver=ver),
                rd1_en=has_src1(spec),
            )
            op.uops_sha[ver] = compiled.sha(ver)
        ops_out.append(op)
    return ops_out[0]


def _build_program():
    import concourse.tile as tile
    from concourse.tile import add_dep_helper
    from concourse import bacc, mybir

    min2 = _register_min2()

    nc = bacc.Bacc("TRN2", target_bir_lowering=False, debug=False)
    f32 = mybir.dt.float32
    f16 = mybir.dt.float16

    KR = 32 + K   # replicated operand stack height (rows 0..12 and 32..44)
    lhs1_d = nc.dram_tensor("lhs1", [KR, ROWS_PER_CORE], f16, kind="ExternalInput")
    rhs2_d = nc.dram_tensor("rhs2", [KR, M], f16, kind="ExternalInput")
    lhs2_d = nc.dram_tensor("lhs2", [KR, ROWS_PER_CORE], f16, kind="ExternalInput")
    rhs1_d = nc.dram_tensor("rhs1", [KR, N], f16, kind="ExternalInput")
    out_d = nc.dram_tensor("out", [128, 2 * BLOCKS], f32, kind="ExternalOutput")

    with tile.TileContext(nc) as tc:
        with (
            tc.tile_pool(name="ops", bufs=1) as ops,
            tc.tile_pool(name="ps_keep", bufs=2, space="PSUM") as ps_keep,
            tc.tile_pool(name="ps_copy", bufs=2, space="PSUM") as ps_copy,
            tc.tile_pool(name="scopy", bufs=5) as scopy,
            tc.tile_pool(name="scratch", bufs=3) as scratch_pool,
            tc.tile_pool(name="small", bufs=1) as small,
        ):
            # Operand stacks come pre-replicated from the host at base
            # partitions 0 and 32, so consecutive matmuls can target
            # different PE row-groups: LDWEIGHTS for one row-group overlaps
            # the matmul streaming in the other, and alternating-group
            # matmul pairs stream concurrently.  The first 2048 columns
            # live in separate "early" tiles: tensor dependencies are
            # tile-granular, so the first matmuls start as soon as the
            # small early DMAs land.  Early pieces ride the otherwise-idle
            # scalar/gpsimd DMA queues (consumers wait on per-queue
            # completion counts, so short queues mean early starts).
            E = 4 * CHUNK
            lhs1 = ops.tile([KR, ROWS_PER_CORE], f16, tag="lhs1")
            lhs2 = ops.tile([KR, ROWS_PER_CORE], f16, tag="lhs2")
            rhs2e = ops.tile([KR, E], f16, tag="rhs2e")
            rhs2 = ops.tile([KR, M - E], f16, tag="rhs2")
            rhs1e = ops.tile([KR, E], f16, tag="rhs1e")
            rhs1 = ops.tile([KR, N - E], f16, tag="rhs1")
            nc.scalar.dma_start(lhs1[:], lhs1_d[:])
            nc.scalar.dma_start(rhs2e[:], rhs2_d[:, 0:E])
            nc.gpsimd.dma_start(lhs2[:], lhs2_d[:])
            nc.gpsimd.dma_start(rhs1e[:], rhs1_d[:, 0:E])
            for s in (slice(E, 4096), slice(4096, M)):
                d = slice(s.start - E, s.stop - E)
                nc.sync.dma_start(rhs2[:, d], rhs2_d[:, s])
                nc.sync.dma_start(rhs1[:, d], rhs1_d[:, s])

            rowmin = small.tile([128, 2 * BLOCKS], f32, tag="rowmin")
            rowpart = small.tile([128, 2 * BLOCKS, N_CHUNKS // 4], f32, tag="rowpart")

            # One block in flight (PSUM double-buffers across quads), but
            # consecutive matmuls alternate between the two operand replicas
            # (PE row-groups 0 and 32), so pairs of matmuls stream
            # concurrently — 2 cols/cycle.  That matters because the HAM
            # clock never leaves 1.2 GHz in this environment.  Per quad: two
            # chunks land in a 2-bank PSUM tile the fused MIN2 op reads via
            # the PSUM port; the next two are copied to SBUF by ScalarE and
            # feed the SBUF port, so each MIN2 consumes 2 elements/cycle.
            # The block's running min chains through `cell`.
            for o, (lhs_s, rhs_e, rhs_r) in enumerate(
                ((lhs1, rhs2e, rhs2), (lhs2, rhs1e, rhs1))
            ):
                for b in range(BLOCKS):
                    ob = o * BLOCKS + b
                    bc = slice(b * 128, (b + 1) * 128)
                    for q in range(N_CHUNKS // 4):
                        rhs_s = rhs_e if q == 0 else rhs_r
                        j0 = 4 * q * CHUNK - (0 if q == 0 else E)
                        pk = ps_keep.tile([128, 2, CHUNK], f32, name="pk", tag="pk")
                        pc = ps_copy.tile([128, 2, CHUNK], f32, name="pc", tag="pc")
                        for t, dst in ((0, pk), (1, pk), (2, pc), (3, pc)):
                            g = 32 * (t % 2)
                            nc.tensor.matmul(
                                dst[:, t % 2, :],
                                lhs_s[g : g + K, bc],
                                rhs_s[g : g + K, j0 + t * CHUNK : j0 + (t + 1) * CHUNK],
                            )
                        sc = scopy.tile([128, 2, CHUNK], f32, name="sc", tag="sc")
                        nc.scalar.copy(sc[:], pc[:])
                        scr = scratch_pool.tile([128, 2, CHUNK], f32, name="scr", tag="scr")
                        nc.vector._custom_dve(
                            min2,
                            out=scr[:],
                            in0=pk[:],
                            in1=sc[:],
                            s0=FP32_MAX,
                            accum_out=rowpart[:, ob, q : q + 1],
                        )

            nc.vector.tensor_reduce(
                rowmin[:],
                rowpart[:],
                axis=mybir.AxisListType.X,
                op=mybir.AluOpType.min,
            )
            nc.sync.dma_start(out_d[:], rowmin[:])

    nc.compile()
    return nc


def _get_program():
    global _compiled
    if _compiled is None:
        _compiled = _build_program()
    return _compiled


def _split16(v):
    """fp64 vector -> (hi, lo) fp16 with v ~= hi + lo to ~2^-22 rel."""
    hi = v.astype(np.float16)
    lo = (v - hi.astype(np.float64)).astype(np.float16)
    return hi.astype(np.float64), lo.astype(np.float64)


def _replicate(stack):
    """[13, n] -> [45, n] with copies at rows 0..12 and 32..44 (PE
    row-groups 0 and 32); filler rows are zeros."""
    out = np.zeros((32 + K, stack.shape[1]), dtype=stack.dtype)
    out[0:K] = stack
    out[32 : 32 + K] = stack
    return np.ascontiguousarray(out)


def _aug_operands(s):
    """Build [45, n] lhsT and rhs operand stacks in fp16 (hi/lo split,
    replicated for the two PE row-groups)."""
    s64 = s.astype(np.float64)
    n = (s64 * s64).sum(axis=1)
    ones = np.ones(s.shape[0], dtype=np.float64)
    xh = [None] * D
    xl = [None] * D
    for d in range(D):
        xh[d], xl[d] = _split16(s64[:, d])
    nh, nl = _split16(n)
    lhs = np.stack(
        [xh[0], xh[1], xh[2], xh[0], xh[1], xh[2], xl[0], xl[1], xl[2],
         nh, nl, ones, ones]
    ).astype(np.float16)
    rhs = np.stack(
        [-2 * xh[0], -2 * xh[1], -2 * xh[2], -2 * xl[0], -2 * xl[1], -2 * xl[2],
         -2 * xh[0], -2 * xh[1], -2 * xh[2], ones, ones, nh, nl]
    ).astype(np.float16)
    return _replicate(lhs), _replicate(rhs)


def _run_device(s1, s2, trace=False):
    from concourse.bass_utils import run_bass_kernel_spmd

    nc = _get_program()
    lhs1_full, rhs1_full = _aug_operands(s1)
    lhs2_full, rhs2_full = _aug_operands(s2)

    in_maps = []
    for r in range(N_CORES):
        sl = slice(r * ROWS_PER_CORE, (r + 1) * ROWS_PER_CORE)
        in_maps.append(
            {
                "lhs1": np.ascontiguousarray(lhs1_full[:, sl]),
                "rhs2": rhs2_full,
                "lhs2": np.ascontiguousarray(lhs2_full[:, sl]),
                "rhs1": rhs1_full,
            }
        )

    res = run_bass_kernel_spmd(nc, in_maps, list(range(N_CORES)), trace=trace)

    d1min = np.concatenate(
        [res.results[r]["out"][:, 0:BLOCKS].T.reshape(-1) for r in range(N_CORES)]
    )
    d2min = np.concatenate(
        [res.results[r]["out"][:, BLOCKS : 2 * BLOCKS].T.reshape(-1) for r in range(N_CORES)]
    )
    return d1min, d2min, res


def kernel(set1, set2, hausdorff=0, w_set1_set2=1, w_set2_set1=1, n_outputs=1):
    s1 = np.ascontiguousarray(np.asarray(set1, dtype=np.float32))
    s2 = np.ascontiguousarray(np.asarray(set2, dtype=np.float32))
    assert s1.shape == (N, D) and s2.shape == (M, D), (s1.shape, s2.shape)
    hausdorff = int(np.asarray(hausdorff))
    w12 = int(np.asarray(w_set1_set2))
    w21 = int(np.asarray(w_set2_set1))
    n_outputs = int(np.asarray(n_outputs))

    d1min, d2min, _ = _run_device(s1, s2)

    d1 = np.sqrt(np.maximum(d1min, 0.0).astype(np.float64))
    d2 = np.sqrt(np.maximum(d2min, 0.0).astype(np.float64))
    reduce = np.mean if hausdorff == 0 else np.max
    t12 = np.float32(reduce(d1)) if w12 != 0 else np.float32(0.0)
    t21 = np.float32(reduce(d2)) if w21 != 0 else np.float32(0.0)

    if n_outputs == 1:
        return np.float32(t12 + t21)
    return (t12, t21)
